# revision 14
# baseline (speedup 1.0000x reference)
"""Single-head causal self-attention on 8 NeuronCores (data-parallel over batch).

Reference computation (per batch element b):
    Q = X @ Wq + bq; K = X @ Wk + bk; V = X @ Wv + bv        # [T, DK]
    S = Q @ K.T / sqrt(DK)  (causal masked)
    out = softmax(S) @ V                                      # [T, DK]

Design (all bf16; fp8 was tested numerically and exceeds the 2e-2 error
budget in every variant):
  - X.T arrives in 4 column-chunks of 512 (ktile-major inside a chunk),
    ASCENDING, each chunk split across the two fast DMA rings
    (sync+scalar) with the weights leading both rings.
  - PE pre-warm: HAM gates the PE clock at 1.2 GHz until ~3.4us of
    sustained activity; dummy matmuls on a zeroed scratch tile bridge
    the DMA wait so real matmuls start at 2.4 GHz.
  - Projections per chunk: pass A [Wv|Wk] -> V.T rows 0:64 / K.T rows
    64:128; pass B [Wq|Wq] (duplicated for base-partition match),
    interleaved at half-k granularity.  Drains on DVE; the Scalar (Act)
    engine does EXP ONLY.
  - Attention is BLOCK-MAJOR (t-major): for each 512-column t-block b
    (ascending, unlocked by chunk b's projection), all s-tile pairs
    p = 0..min(2b+1,7) run scores -> one merged exp over a [128,2,512]
    PSUM tile -> P@[V|1] accumulation into o(b).  Only one o-tile
    accumulates at a time (bufs=2 rotation), so PSUM is
    2 proj + 2x2 scores + 2 out = 8 banks, and each block's output
    drains as soon as its last pair finishes (staggered out-DMA).
  - Causality: pair p contributes to blocks b >= p//2 only; the
    diagonal unit gets memset zero-fill + one tri-mask multiply per
    slot after the exp.
  - The ones column appended to the V stationaries makes the P@V
    matmul also produce the softmax denominator (row 64).
  - Device output per core: [65, T]; host computes (O_unnorm / l).T.
"""

import sys

sys.path.insert(0, "/opt/trn_rl_repo")

import numpy as np
import ml_dtypes

B, T, C, DK = 8, 2048, 1024, 64
KT = C // 128            # 8 k-tiles in the contraction over C
NS = T // 128            # 16 s-tiles
NCHUNK = T // 512        # 4 chunks of 512
NP = NS // 2             # 8 s-tile pairs
SCALE = 1.0 / np.sqrt(DK)
BF16 = np.dtype(ml_dtypes.bfloat16)

_CACHE = {}


def _build():
    from concourse import bass, bacc, tile

    mybir = bass.mybir
    f32 = mybir.dt.float32
    bf16 = mybir.dt.bfloat16

    nc = bacc.Bacc(
        "TRN2", target_bir_lowering=False, debug=False, num_devices=B
    )

    xc_d = [
        nc.dram_tensor(f"xc{c}", [128, KT * 512], bf16, kind="ExternalInput")
        for c in range(NCHUNK)
    ]
    wvk_d = nc.dram_tensor("wvk", [128, KT * 128], bf16, kind="ExternalInput")
    wqq_d = nc.dram_tensor("wqq", [128, KT * 128], bf16, kind="ExternalInput")
    bvk_d = nc.dram_tensor("bvk", [128, 1], f32, kind="ExternalInput")
    bqq_d = nc.dram_tensor("bqq", [128, 1], f32, kind="ExternalInput")
    out_d = nc.dram_tensor("out", [65, T], bf16, kind="ExternalOutput")

    # packed consts: cols 0:128 upper-tri mask, 128:192 identity (rows 0:64)
    cst_np = np.zeros((128, 192), dtype=BF16)
    cst_np[:, 0:128] = np.triu(np.ones((128, 128), dtype=np.float32)).astype(BF16)
    cst_np[0:64, 128:192] = np.eye(64, dtype=np.float32).astype(BF16)
    cst_d = nc.inline_tensor(cst_np, "cst")

    EXP = mybir.ActivationFunctionType.Exp

    with tile.TileContext(nc) as tc:
        with tc.tile_pool(name="const", bufs=1) as cpool, \
             tc.tile_pool(name="weights", bufs=1) as wpool, \
             tc.tile_pool(name="x", bufs=1) as xpool, \
             tc.tile_pool(name="acts", bufs=1) as apool, \
             tc.tile_pool(name="et", bufs=4) as etpool, \
             tc.tile_pool(name="pp", bufs=1, space="PSUM") as pp, \
             tc.tile_pool(name="pst", bufs=2, space="PSUM") as pst, \
             tc.tile_pool(name="po", bufs=2, space="PSUM") as po:

            # ---- PE pre-warm (emitted first: no DMA dependencies) ----
            warm_in = cpool.tile([128, 256], bf16, name="warm_in")
            nc.gpsimd.memset(warm_in[:], 0.0)
            for w in range(28):
                wps = pst.tile([128, 256], f32, tag="st", name="warm_ps")
                nc.tensor.matmul(
                    wps[:], warm_in[:, 0:128], warm_in[:],
                    start=True, stop=True,
                )

            # ---- DMAs: weights lead the two fast rings (sync+scalar) ----
            wvk = wpool.tile([128, KT * 128], bf16)
            nc.sync.dma_start(out=wvk[:], in_=wvk_d[:])
            wqq = wpool.tile([128, KT * 128], bf16)
            nc.scalar.dma_start(out=wqq[:], in_=wqq_d[:])
            cst = cpool.tile([128, 192], bf16)
            nc.gpsimd.dma_start(out=cst[:], in_=cst_d[:])
            tri = cst[:, 0:128]
            ident = cst[0:64, 128:192]
            bvk = cpool.tile([128, 1], f32)
            nc.gpsimd.dma_start(out=bvk[:], in_=bvk_d[:])
            bqq = cpool.tile([128, 1], f32)
            nc.gpsimd.dma_start(out=bqq[:], in_=bqq_d[:])

            # X chunks ascending, each split across both rings
            xs = [None] * NCHUNK
            half = KT * 512 // 2
            for c in range(NCHUNK):
                xk = xpool.tile([128, KT * 512], bf16, tag=f"x{c}")
                nc.sync.dma_start(out=xk[:, 0:half], in_=xc_d[c][:, 0:half])
                nc.scalar.dma_start(
                    out=xk[:, half:2 * half], in_=xc_d[c][:, half:2 * half]
                )
                xs[c] = xk

            # persistent activations
            vk = apool.tile([128, T], bf16, tag="vk")   # V.T 0:64 | K.T 64:128
            qq = apool.tile([128, T], bf16, tag="qq")   # Q.T duplicated
            v1 = apool.tile([128, NS * 65], bf16, tag="v1")  # [V_i | 1]
            osb = apool.tile([65, T], bf16, tag="osb")

            nc.gpsimd.memset(v1[:], 1.0)

            def proj_chunk(c):
                sl = slice(512 * c, 512 * (c + 1))
                psA = pp.tile([128, 512], f32, tag="psA", name="psA")
                psB = pp.tile([128, 512], f32, tag="psB", name="psB")
                for ps, w in ((psA, wvk), (psB, wqq)):
                    for k in range(KT // 2):
                        nc.tensor.matmul(
                            ps[:],
                            w[:, 128 * k:128 * (k + 1)],
                            xs[c][:, 512 * k:512 * (k + 1)],
                            start=(k == 0), stop=False,
                        )
                for ps, w in ((psA, wvk), (psB, wqq)):
                    for k in range(KT // 2, KT):
                        nc.tensor.matmul(
                            ps[:],
                            w[:, 128 * k:128 * (k + 1)],
                            xs[c][:, 512 * k:512 * (k + 1)],
                            start=False, stop=(k == KT - 1),
                        )
                nc.vector.tensor_scalar_add(vk[:, sl], psA[:], bvk[:])
                nc.vector.tensor_scalar_add(qq[:, sl], psB[:], bqq[:])
                for i in range(4 * c, 4 * c + 4):
                    vt = pp.tile([128, 64], bf16, tag="psB", name="vt")
                    nc.tensor.transpose(
                        vt[:], vk[0:64, 128 * i:128 * (i + 1)], ident[:]
                    )
                    nc.vector.tensor_copy(v1[:, 65 * i:65 * i + 64], vt[:])

            def attn_unit(p, b, otile):
                """Scores+exp+mask+PV for s-tile pair p on t-block b."""
                i0, i1 = 2 * p, 2 * p + 1
                ts1 = 128 * i1
                diag = (b == p // 2)
                s0 = max(128 * i0, 512 * b)
                o0 = s0 - 512 * b
                st = pst.tile([128, 2, 512], f32, tag="st", name="st")
                for u, it in ((0, i0), (1, i1)):
                    nc.tensor.matmul(
                        st[:, u, o0:512],
                        vk[64:128, 128 * it:128 * (it + 1)],
                        qq[64:128, s0:512 * (b + 1)],
                        start=True, stop=True,
                    )
                etp = etpool.tile([128, 2, 512], bf16, tag="et", name="etp")
                nc.scalar.activation(
                    etp[:, :, o0:512], st[:, :, o0:512], EXP, scale=SCALE
                )
                if diag:
                    o1 = ts1 - 512 * b
                    if o0 > 0:
                        nc.gpsimd.memset(etp[:, 0, 0:o0], 0.0)
                    nc.gpsimd.memset(etp[:, 1, 0:o1], 0.0)
                    nc.vector.tensor_mul(
                        etp[:, 0, o0:o0 + 128], etp[:, 0, o0:o0 + 128], tri[:]
                    )
                    nc.vector.tensor_mul(
                        etp[:, 1, o1:o1 + 128], etp[:, 1, o1:o1 + 128], tri[:]
                    )
                pmax = min(2 * b + 1, NP - 1)
                for u, it in ((0, i0), (1, i1)):
                    nc.tensor.matmul(
                        otile[:],
                        v1[:, 65 * it:65 * it + 65],
                        etp[:, u, 0:512],
                        start=(p == 0 and u == 0),
                        stop=(p == pmax and u == 1),
                    )

            # ---- block-major schedule: chunk b unlocks t-block b ----
            for b in range(NCHUNK):
                proj_chunk(b)
                otile = po.tile([65, 512], f32, tag="o", name=f"o{b}")
                for p in range(min(2 * b + 1, NP - 1) + 1):
                    attn_unit(p, b, otile)
                sl = slice(512 * b, 512 * (b + 1))
                nc.vector.tensor_copy(osb[:, sl], otile[:])
                nc.sync.dma_start(out=out_d[:, sl], in_=osb[:, sl])

    nc.compile()
    return nc


def _get_nc():
    if "nc" not in _CACHE:
        _CACHE["nc"] = _build()
    return _CACHE["nc"]


def make_in_maps(X, Wq, bq, Wk, bk, Wv, bv):
    X = np.asarray(X, dtype=np.float32)
    Wq = np.asarray(Wq, dtype=np.float32)
    Wk = np.asarray(Wk, dtype=np.float32)
    Wv = np.asarray(Wv, dtype=np.float32)
    bq = np.asarray(bq, dtype=np.float32)
    bk = np.asarray(bk, dtype=np.float32)
    bv = np.asarray(bv, dtype=np.float32)

    wvk = np.ascontiguousarray(
        np.concatenate([Wv, Wk], axis=1).reshape(KT, 128, 128)
        .transpose(1, 0, 2).reshape(128, KT * 128)
    ).astype(BF16)
    wqq = np.ascontiguousarray(
        np.concatenate([Wq, Wq], axis=1).reshape(KT, 128, 128)
        .transpose(1, 0, 2).reshape(128, KT * 128)
    ).astype(BF16)
    bvk = np.concatenate([bv, bk]).reshape(128, 1).astype(np.float32)
    bqq = np.concatenate([bq, bq]).reshape(128, 1).astype(np.float32)

    in_maps = []
    for b in range(B):
        xt = X[b].T.astype(BF16)          # [C, T]
        m = {"wvk": wvk, "wqq": wqq, "bvk": bvk, "bqq": bqq}
        for c in range(NCHUNK):
            blk = xt[:, 512 * c:512 * (c + 1)]          # [1024, 512]
            m[f"xc{c}"] = np.ascontiguousarray(
                blk.reshape(KT, 128, 512).transpose(1, 0, 2).reshape(128, KT * 512)
            )
        in_maps.append(m)
    return in_maps


def kernel(X, Wq, bq, Wk, bk, Wv, bv):
    from concourse.bass_utils import run_bass_kernel_spmd

    nc = _get_nc()
    in_maps = make_in_maps(X, Wq, bq, Wk, bk, Wv, bv)
    res = run_bass_kernel_spmd(nc, in_maps, list(range(B)))

    out = np.empty((B, T, DK), dtype=np.float32)
    for b in range(B):
        r = np.asarray(res.results[b]["out"], dtype=np.float32)
        out[b] = (r[:64] / r[64:65]).T
    return out
